# revision 5
# baseline (speedup 1.0000x reference)
"""MoE (T=2048 H=2048 I=1408 E=16 top-2) on 8 trn2 NeuronCores.

Strategy (expert-parallel, per the sharding hint):
  - Router (gate linear + top-2 sigmoid + renorm) computed on host in f64
    (matches the f32 reference's top-k selections with margin to spare).
  - Tokens are dispatched host-side: experts sorted by load; the 8 heaviest
    go in slot 0 (capacity C0 = max load rounded to 2) and the 8 lightest in
    slot 1 (capacity C1), one of each per core.
  - Each core runs a Bass/Tile kernel computing, per owned expert:
        gT = Wg_e.T @ X_e.T   uT = Wu_e.T @ X_e.T        (bf16 matmul)
        hT = silu(gT) * uT                                (fp32, cast bf16)
        yT = Wd_e.T(row-tiles) @ hT                       (bf16 out, unscaled)
    fp32 accumulation stays in PSUM.  Outputs are bf16 and unscaled; the
    per-token combine weights are applied host-side during the gather.
  - DMA layout tuned for the two HW-DGE rings (sync=q1, scalar=q10):
      * scalar ring carries the gate+up weight stream (one fused [P,2,H]
        slab per i-tile, 4KB elements);
      * sync ring carries xt (packed [P,HT,C] per expert so chunks have
        2.3-4.6KB elements), the down-proj slabs, and the bf16 outputs.
  - Host combines: out[t] = w0*Y[:,col0(t)] + w1*Y[:,col1(t)].
"""

import numpy as np
import ml_dtypes

import concourse.bacc as bacc
import concourse.mybir as mybir
import concourse.tile as tile
from concourse.bass_utils import run_bass_kernel_spmd

T = 2048
H = 2048
I = 1408
E = 16
K = 2
NCORES = 8
EPC = E // NCORES  # experts per core (2)
P = 128
HT = H // P        # 16 h-tiles
IT = I // P        # 11 i-tiles

BF16 = mybir.dt.bfloat16
F32 = mybir.dt.float32
nbf = ml_dtypes.bfloat16

_kernel_cache: dict[tuple, tuple] = {}
_weight_cache: dict[tuple, tuple] = {}

# xt chunk spans per slot (h-tile ranges); first chunks small so the first
# gate matmuls can start early, later chunks big for DMA elem efficiency.
XT_CHUNKS = ([(0, 4), (4, 8), (8, 16)], [(0, 8), (8, 16)])
# output DMA chunk spans (h-tiles per DMA)
OUT_CHUNKS = [(0, 4), (4, 8), (8, 12), (12, 16)]


def _build(caps: tuple[int, ...]):
    """Build + compile the per-core kernel for slot capacities `caps`
    (each a multiple of 2, <= 512)."""
    Cm = max(caps)

    nc = bacc.Bacc("TRN2", target_bir_lowering=False, debug=False, num_devices=NCORES)
    # activations, token-gathered+transposed per slot, packed so that a
    # multi-h-tile chunk is contiguous per partition:
    #   xt{j}[p, h, k] = X[token_k_of_slot_j, h*P + p]
    xts_d = [
        nc.dram_tensor(f"xt{j}", [P, HT, caps[j]], BF16, kind="ExternalInput")
        for j in range(EPC)
    ]
    # fused gate+up weight slabs (4KB rows):
    #   wgu[e, i, 0, p, h*P + c] = Wg[e, h*P + p, i*P + c]
    #   wgu[e, i, 1, p, h*P + c] = Wu[e, h*P + p, i*P + c]
    wgu = nc.dram_tensor("wgu", [EPC, IT, 2, P, H], BF16, kind="ExternalInput")
    #   wds[e, i, p, :] = Wd[e, i*P + p, :]
    wds = nc.dram_tensor("wds", [EPC, IT, P, H], BF16, kind="ExternalInput")
    # unscaled bf16 expert outputs:  yo{j}[p, h, k] = y_j[h*P + p, token_k]
    yos_d = [
        nc.dram_tensor(f"yo{j}", [P, HT, caps[j]], BF16, kind="ExternalOutput")
        for j in range(EPC)
    ]

    with tile.TileContext(nc) as tc:
        with (
            tc.tile_pool(name="xt0_pool", bufs=len(XT_CHUNKS[0])) as xt0_pool,
            tc.tile_pool(name="xt1_pool", bufs=len(XT_CHUNKS[1])) as xt1_pool,
            tc.tile_pool(name="w00_pool", bufs=2) as w00_pool,
            tc.tile_pool(name="wgu_pool", bufs=4) as wgu_pool,
            tc.tile_pool(name="wd_pool", bufs=IT + 2) as wd_pool,
            tc.tile_pool(name="ht_pool", bufs=IT + 2) as ht_pool,
            tc.tile_pool(name="tmp_pool", bufs=2) as tmp_pool,
            tc.tile_pool(name="out_pool", bufs=3) as out_pool,
            tc.tile_pool(name="pg_pool", bufs=2, space="PSUM") as pg_pool,
            tc.tile_pool(name="pu_pool", bufs=2, space="PSUM") as pu_pool,
            tc.tile_pool(name="py_pool", bufs=4, space="PSUM") as py_pool,
        ):
            # --- startup issues -------------------------------------------
            # scalar ring (q10): first expert's gate slab then up slab (the
            # fused slab would delay the first gate matmul), then the fused
            # wgu stream (emitted inside the i-loop below).
            wg00 = w00_pool.tile([P, H], BF16, name="wg00")
            nc.scalar.dma_start(wg00[:], wgu.ap()[0, 0, 0])
            wu00 = w00_pool.tile([P, H], BF16, name="wu00")
            nc.scalar.dma_start(wu00[:], wgu.ap()[0, 0, 1])
            # sync ring (q1): xt chunks for slot 0 then slot 1.
            xt_tiles: list[list] = []
            for j, pool in enumerate((xt0_pool, xt1_pool)):
                tiles = []
                for ci, (h0, h1) in enumerate(XT_CHUNKS[j]):
                    t_ = pool.tile(
                        [P, h1 - h0, caps[j]], BF16, name=f"xt{j}c{ci}"
                    )
                    nc.sync.dma_start(t_[:], xts_d[j].ap()[:, h0:h1, :])
                    tiles.append((h0, h1, t_))
                xt_tiles.append(tiles)

            def xt_slice(j, h):
                for h0, h1, t_ in xt_tiles[j]:
                    if h0 <= h < h1:
                        return t_[:, h - h0, :]
                raise AssertionError

            for e in range(EPC):
                C = caps[e]
                # down-proj slabs for this expert on the sync ring; issued
                # up-front so they stream during the gate/up phase.
                wd_tiles = []
                for i in range(IT):
                    wd_t = wd_pool.tile([P, H], BF16, tag="wd")
                    nc.sync.dma_start(wd_t[:], wds.ap()[e, i])
                    wd_tiles.append(wd_t)

                # ---- gate/up projections, one fused slab per i-tile ----
                hts = []
                for i in range(IT):
                    if e == 0 and i == 0:
                        g_sl = lambda h: wg00[:, h * P:(h + 1) * P]
                        u_sl = lambda h: wu00[:, h * P:(h + 1) * P]
                    else:
                        wgu_t = wgu_pool.tile([P, 2, H], BF16, tag="wgu")
                        nc.scalar.dma_start(
                            wgu_t[:], wgu.ap()[e, i].rearrange("g p h -> p g h")
                        )
                        g_sl = lambda h, t_=wgu_t: t_[:, 0, h * P:(h + 1) * P]
                        u_sl = lambda h, t_=wgu_t: t_[:, 1, h * P:(h + 1) * P]
                    pg = pg_pool.tile([P, Cm], F32, tag="pg")
                    pu = pu_pool.tile([P, Cm], F32, tag="pu")
                    for h in range(HT):
                        nc.tensor.matmul(
                            pg[:, :C], g_sl(h), xt_slice(e, h),
                            start=(h == 0), stop=(h == HT - 1),
                        )
                    for h in range(HT):
                        nc.tensor.matmul(
                            pu[:, :C], u_sl(h), xt_slice(e, h),
                            start=(h == 0), stop=(h == HT - 1),
                        )
                    tmp = tmp_pool.tile([P, Cm], F32, tag="tmp")
                    nc.scalar.activation(
                        tmp[:, :C], pg[:, :C], mybir.ActivationFunctionType.Silu
                    )
                    ht_t = ht_pool.tile([P, Cm], BF16, tag="ht")
                    nc.vector.tensor_tensor(
                        ht_t[:, :C], tmp[:, :C], pu[:, :C], mybir.AluOpType.mult
                    )
                    hts.append(ht_t)

                # ---- down projection (transposed bf16 output) ----
                oc = None
                for h in range(HT):
                    py = py_pool.tile([P, Cm], F32, tag="py")
                    for i in range(IT):
                        nc.tensor.matmul(
                            py[:, :C],
                            wd_tiles[i][:, h * P:(h + 1) * P],
                            hts[i][:, :C],
                            start=(i == 0), stop=(i == IT - 1),
                        )
                    if h % 4 == 0:
                        oc = out_pool.tile([P, 4, Cm], BF16, tag="oc")
                    nc.vector.tensor_scalar_mul(oc[:, h % 4, :C], py[:, :C], 1.0)
                    if h % 4 == 3:
                        nc.sync.dma_start(
                            yos_d[e].ap()[:, h - 3:h + 1, :], oc[:, :, :C]
                        )

    nc.compile()
    return nc


def _get_kernel(caps):
    if caps not in _kernel_cache:
        _kernel_cache[caps] = _build(caps)
    return _kernel_cache[caps]


def _prep_weights(w_gate_proj, w_up_proj, w_down_proj):
    key = tuple(
        (a.__array_interface__["data"][0], a.shape)
        for a in (w_gate_proj, w_up_proj, w_down_proj)
    )
    if key in _weight_cache:
        return _weight_cache[key]
    wg_bf = np.asarray(w_gate_proj, np.float32).astype(nbf)  # [E, H, I]
    wu_bf = np.asarray(w_up_proj, np.float32).astype(nbf)    # [E, H, I]
    wd_bf = np.asarray(w_down_proj, np.float32).astype(nbf)  # [E, I, H]
    wg_slab = wg_bf.reshape(E, HT, P, IT, P).transpose(0, 3, 2, 1, 4).reshape(E, IT, P, H)
    wu_slab = wu_bf.reshape(E, HT, P, IT, P).transpose(0, 3, 2, 1, 4).reshape(E, IT, P, H)
    wgu_all = np.ascontiguousarray(np.stack([wg_slab, wu_slab], axis=2))  # [E, IT, 2, P, H]
    wd_rows = np.ascontiguousarray(wd_bf.reshape(E, IT, P, H))
    _weight_cache.clear()
    _weight_cache[key] = (wgu_all, wd_rows)
    return _weight_cache[key]


def _route(X, WG):
    """f64 replica of the reference router; returns per-expert dispatch."""
    logits = X.astype(np.float64) @ np.asarray(WG, np.float64)
    scores = 1.0 / (1.0 + np.exp(-logits))
    top2 = np.argsort(-scores, axis=1, kind="stable")[:, :K]
    w = np.take_along_axis(scores, top2, 1)
    wn = (w / w.sum(1, keepdims=True)).astype(np.float32)
    tok_list, w_list = [], []
    for e in range(E):
        hit = top2 == e  # [T, K]
        tok = np.nonzero(hit.any(1))[0]
        kk = hit[tok, 1].astype(np.int64)
        tok_list.append(tok)
        w_list.append(wn[tok, kk])
    return tok_list, w_list


def _pack_xt(X, tokens, cap):
    """[P, HT, cap] bf16: xt[p, h, k] = X[tokens[k], h*P + p]."""
    n = len(tokens)
    Xg = np.zeros((cap, H), np.float32)
    Xg[:n] = X[tokens]
    arr = Xg.T.reshape(HT, P, cap).transpose(1, 0, 2)
    return np.ascontiguousarray(arr.astype(nbf))


def _run(inputs: dict, trace: bool = False, trace_cores=None):
    X = np.ascontiguousarray(np.asarray(inputs["hidden_states"], np.float32))
    tok_list, w_list = _route(X, inputs["w_gate"])
    counts = np.array([len(t) for t in tok_list])

    # slot assignment: heaviest 8 experts in slot 0, lightest 8 in slot 1
    order = np.argsort(-counts, kind="stable")
    slot_exp = [order[:NCORES], order[NCORES:]]  # [slot][core] -> expert
    caps = tuple(
        min(512, max(16, int(-(-counts[slot_exp[j]].max() // 2)) * 2))
        for j in range(EPC)
    )
    if counts.max() > 512:
        raise RuntimeError(f"expert load {counts.max()} exceeds supported capacity")
    nc = _get_kernel(caps)
    wgu_all, wd_rows = _prep_weights(
        inputs["w_gate_proj"], inputs["w_up_proj"], inputs["w_down_proj"]
    )

    in_maps = []
    for c in range(NCORES):
        experts = [int(slot_exp[j][c]) for j in range(EPC)]
        m = {
            "wgu": np.ascontiguousarray(wgu_all[experts]),
            "wds": np.ascontiguousarray(wd_rows[experts]),
        }
        for j, e in enumerate(experts):
            m[f"xt{j}"] = _pack_xt(X, tok_list[e], caps[j])
        in_maps.append(m)

    if trace:
        _install_trace_shim()
    res = run_bass_kernel_spmd(
        nc,
        in_maps,
        core_ids=list(range(NCORES)),
        trace=trace,
        trace_cores=trace_cores,
    )

    # combine on host: out[t] = w0*Y[:, col0] + w1*Y[:, col1]
    # big: [H, NCORES*(C0+C1)] in (core, slot) column order
    col_blocks = []
    for c in range(NCORES):
        for j in range(EPC):
            y = np.asarray(res.results[c][f"yo{j}"], np.float32)  # [P, HT, Cj]
            col_blocks.append(y.transpose(1, 0, 2).reshape(H, caps[j]))
    big = np.concatenate(col_blocks, axis=1)
    TCc = sum(caps)

    col_a = np.full(T, -1, np.int64)
    col_b = np.full(T, -1, np.int64)
    w_a = np.zeros(T, np.float32)
    w_b = np.zeros(T, np.float32)
    for j in range(EPC):
        base_j = sum(caps[:j])
        for c in range(NCORES):
            e = int(slot_exp[j][c])
            tok = tok_list[e]
            cols = c * TCc + base_j + np.arange(counts[e])
            first = col_a[tok] < 0
            col_a[tok[first]] = cols[first]
            w_a[tok[first]] = w_list[e][first]
            col_b[tok[~first]] = cols[~first]
            w_b[tok[~first]] = w_list[e][~first]
    assert (col_a >= 0).all() and (col_b >= 0).all()
    out = (big[:, col_a] * w_a[None, :] + big[:, col_b] * w_b[None, :]).T
    return np.ascontiguousarray(out.astype(np.float32)), res


def kernel(**inputs) -> np.ndarray:
    out, _ = _run(inputs, trace=False)
    return out


def _install_trace_shim():
    """Make run_bass_kernel_spmd(trace=True) work under axon: register the
    NTFF profile hook that the slim agent image's antenv stub lacks."""
    import sys, types

    if "antenv.axon_hooks" not in sys.modules:
        import antenv

        mod = types.ModuleType("antenv.axon_hooks")
        mod._hook = None
        mod.set_axon_ntff_profile_hook = lambda h: setattr(mod, "_hook", h)
        mod.get_axon_ntff_profile_hook = lambda: mod._hook
        sys.modules["antenv.axon_hooks"] = mod
        antenv.axon_hooks = mod
    if sys.modules["antenv.axon_hooks"].get_axon_ntff_profile_hook() is None:
        from trn_agent_boot.trn_boot import _ntff_profile_via_ctypes

        sys.modules["antenv.axon_hooks"].set_axon_ntff_profile_hook(
            _ntff_profile_via_ctypes("/opt/axon/libaxon_pjrt.so")
        )
